# revision 6
# baseline (speedup 1.0000x reference)
"""DirichletLoss kernel for 8 trn2 NeuronCores.

Math: per graph b, per channel d:
    de[d] = f_d^T L f_d  with f = row-normalized h.
A scalar equals its transpose, so f_d^T L f_d == f_d^T L^T f_d. We exploit
this to compute Mf = L^T @ f with L in its NATURAL row-major layout as the
matmul stationary operand (lhsT[K=i, M=j] = L[i, j]) and f (natural layout)
as the moving operand. No transpose of the big L matrices anywhere.

Sharding: graph b -> core b. Each core handles laplacian_s[b] and
laplacian_t[b] (16 MiB each), streaming them through the PE while
accumulating Mf in PSUM, then a multiply-reduce produces a [128, 2]
partial per core. Host finishes the (tiny) masked mean.
"""

import numpy as np

import concourse.bacc as bacc
import concourse.bass as bass
import concourse.mybir as mybir
import concourse.tile as tile
from concourse.bass_utils import run_bass_kernel_spmd

B = 8
N = 2048
D = 64
P = 128
NCHUNK = N // P  # 16
F32 = mybir.dt.float32

# --- tunables -------------------------------------------------------------
SLAB_ROWS = 256          # rows of L per DMA (must be multiple of 128)
SLAB_BUFS = 3            # slab pool double/triple buffering
# --------------------------------------------------------------------------


def _emit_body(nc, tc, pools, aps):
    fpool, slabpool, psumpool, smallpool, outpool = pools
    Ls, hs, Lt, ht, out = aps
    n_blk = SLAB_ROWS // P

    out_sb = outpool.tile([P, 2], F32, tag="out_sb")

    for side, (L_ap, h_ap) in enumerate(((Ls, hs), (Lt, ht))):
        # ---- load h, packed: f_raw[p, k*64+d] = h[k*128+p, d] ----
        f_raw = fpool.tile([P, NCHUNK * D], F32, tag="f_raw")
        nc.sync.dma_start(
            out=f_raw[:], in_=h_ap.rearrange("(k p) d -> p k d", p=P)
        )

        # ---- row L2 norms per (p, k) segment ----
        sq = smallpool.tile([P, NCHUNK * D], F32, tag="sq")
        nc.scalar.square(sq[:], f_raw[:])
        ss = smallpool.tile([P, NCHUNK], F32, tag="ss")
        nc.vector.reduce_sum(
            out=ss[:],
            in_=sq[:].rearrange("p (k d) -> p k d", d=D),
            axis=mybir.AxisListType.X,
        )
        nrm = smallpool.tile([P, NCHUNK], F32, tag="nrm")
        nc.scalar.sqrt(nrm[:], ss[:])
        nc.vector.tensor_scalar_max(nrm[:], nrm[:], 1e-12)
        inv = smallpool.tile([P, NCHUNK], F32, tag="inv")
        nc.vector.reciprocal(inv[:], nrm[:])

        # ---- f = h / max(||h||, eps), per chunk ----
        f_all = fpool.tile([P, NCHUNK * D], F32, tag="f_all")
        for k in range(NCHUNK):
            nc.scalar.mul(
                f_all[:, k * D : (k + 1) * D],
                f_raw[:, k * D : (k + 1) * D],
                inv[:, k : k + 1],
            )

        # ---- Mf = L^T @ f, accumulated over row-slabs ----
        psum = psumpool.tile([P, NCHUNK * D], F32, tag="psum")
        for g in range(N // SLAB_ROWS):
            slab = slabpool.tile([P, n_blk * N], F32, tag="slab")
            nc.sync.dma_start(
                out=slab[:],
                in_=L_ap[g * SLAB_ROWS : (g + 1) * SLAB_ROWS, :].rearrange(
                    "(n p) c -> p n c", p=P
                ),
            )
            for n in range(n_blk):
                k = g * n_blk + n  # global contraction chunk
                for j in range(NCHUNK):
                    nc.tensor.matmul(
                        psum[:, j * D : (j + 1) * D],
                        slab[:, n * N + j * P : n * N + (j + 1) * P],
                        f_all[:, k * D : (k + 1) * D],
                        # PSUM "pending zero" works at bank (2 KiB)
                        # granularity: start only on the first matmul
                        # touching each bank (j=0 and j=8 at k=0);
                        # later first-writes to other j-slices of the
                        # bank overwrite-where-unwritten automatically.
                        start=(k == 0 and j % 8 == 0),
                        stop=(k == NCHUNK - 1 and j % 8 == 7),
                    )

        # ---- r[p] = sum_{k,d} f * Mf ----
        # (tensor_tensor_reduce hard-crashes this HW/ucode build;
        # use separate multiply + reduce instead)
        tmp = smallpool.tile([P, NCHUNK * D], F32, tag="ttr_tmp")
        nc.vector.tensor_tensor(
            out=tmp[:], in0=psum[:], in1=f_all[:], op=mybir.AluOpType.mult
        )
        nc.vector.reduce_sum(
            out=out_sb[:, side : side + 1],
            in_=tmp[:],
            axis=mybir.AxisListType.X,
        )

    nc.sync.dma_start(out=out[:], in_=out_sb[:])


def build_program(reps=1):
    nc = bacc.Bacc(trn_type="TRN2")

    Ls = nc.declare_dram_parameter("Ls", [N, N], F32, isOutput=False)
    hs = nc.declare_dram_parameter("hs", [N, D], F32, isOutput=False)
    Lt = nc.declare_dram_parameter("Lt", [N, N], F32, isOutput=False)
    ht = nc.declare_dram_parameter("ht", [N, D], F32, isOutput=False)
    out = nc.declare_dram_parameter("out", [P, 2], F32, isOutput=True)
    aps = (Ls, hs, Lt, ht, out)

    with tile.TileContext(nc) as tc:
        with (
            tc.tile_pool(name="fpool", bufs=2) as fpool,
            tc.tile_pool(name="slab", bufs=SLAB_BUFS) as slabpool,
            tc.tile_pool(name="psum", bufs=2, space="PSUM") as psumpool,
            tc.tile_pool(name="small", bufs=2) as smallpool,
            tc.tile_pool(name="outp", bufs=2) as outpool,
        ):
            pools = (fpool, slabpool, psumpool, smallpool, outpool)
            if reps == 1:
                _emit_body(nc, tc, pools, aps)
            else:
                with tc.For_i(0, reps, 1):
                    _emit_body(nc, tc, pools, aps)

    nc.compile()
    return nc


_CACHED_NC = None


def _get_nc():
    global _CACHED_NC
    if _CACHED_NC is None:
        _CACHED_NC = build_program()
    return _CACHED_NC


def _shard_inputs(inputs):
    lap_s = np.ascontiguousarray(np.asarray(inputs["laplacian_s"], dtype=np.float32))
    lap_t = np.ascontiguousarray(np.asarray(inputs["laplacian_t"], dtype=np.float32))
    h_s = np.ascontiguousarray(np.asarray(inputs["h_s"], dtype=np.float32))
    h_t = np.ascontiguousarray(np.asarray(inputs["h_t"], dtype=np.float32))
    return [
        {
            "Ls": lap_s[b * N : (b + 1) * N],
            "hs": h_s[b * N : (b + 1) * N],
            "Lt": lap_t[b * N : (b + 1) * N],
            "ht": h_t[b * N : (b + 1) * N],
        }
        for b in range(B)
    ]


def _finish(core_outs, inputs):
    has_s = np.asarray(inputs["has_laplacian_s"]).astype(bool)
    has_t = np.asarray(inputs["has_laplacian_t"]).astype(bool)
    d_s = np.empty(B, dtype=np.float64)
    d_t = np.empty(B, dtype=np.float64)
    for b in range(B):
        o = np.asarray(core_outs[b], dtype=np.float64)
        d_s[b] = o[:, 0].sum() / D
        d_t[b] = o[:, 1].sum() / D
    per_graph = 0.5 * (d_s + d_t)
    valid = np.logical_and(has_s, has_t)
    count = valid.sum()
    total = per_graph[valid].sum()
    value = total / max(count, 1.0) if count > 0 else 0.0
    return np.array(value, dtype=np.float32)


def _run(inputs, trace=False):
    nc = _get_nc()
    in_maps = _shard_inputs(inputs)
    res = run_bass_kernel_spmd(nc, in_maps, list(range(B)), trace=trace)
    out = _finish([res.results[b]["out"] for b in range(B)], inputs)
    return out, res


def kernel(**inputs):
    out, _ = _run(inputs, trace=False)
    return out
